# revision 3
# baseline (speedup 1.0000x reference)
"""GraphTokenPool kernel for Trainium2 (Bass/Tile), data-parallel over batch on 8 cores.

Math (per batch b):
    L = (U @ Vw) @ x + (U @ Vb)        # fused projection, [M, N]
    S = softmax(L, axis=M)             # [M, N]
    T = x @ S^T                        # [C, M]
Returns (T, S) like the reference.

Device layout choices:
  - W^T = (U@Vw)^T [C, M] computed once on device (tiny).
  - L computed in [M, N] orientation (bias per-partition works, big matmul free dim).
  - exp(L) transposed on PE (128-col slices) -> expL^T with N on partitions, so the
    softmax denominator is a free-dim reduce and S^T chunks feed the pooling matmul.
  - x chunks transposed on PE (fp32r transpose-mode) -> x^T tiles for pooling.
  - T accumulated as T^T [M, C] in a single PSUM bank across all N chunks.
  - Device outputs are S^T [N, M] and T^T [M, C]; host transposes (cheap views).
  - Matmuls run in float32r (fast PE path); the S output path stays full fp32.
"""

import numpy as np

import concourse.bass as bass
import concourse.bacc as bacc
import concourse.mybir as mybir
import concourse.tile as tile
import concourse.bass_utils as bass_utils
from concourse.masks import make_identity

F32 = mybir.dt.float32
F32R = mybir.dt.float32r

# Problem shapes (hardcoded per task contract)
B, C, H, W = 32, 384, 64, 64
N = H * W            # 4096
M, R = 64, 16
NCORES = 8
BS = B // NCORES     # 4 batches per core
CS = C // 128        # 3 c-slices
CH = N // 512        # 8 n-chunks of 512
NS = 4               # 4 sub-chunks of 128 per chunk

USE_F32R_MM = True   # float32r for the projection + pooling matmuls
USE_F32R_TR = True   # float32r for the x transposes

MMT = F32R if USE_F32R_MM else F32
TRT = F32R if USE_F32R_TR else F32
# x feeds both the f32r proj matmul and the f32r transposes
XDT = F32R if (USE_F32R_MM or USE_F32R_TR) else F32


def build_program():
    nc = bacc.Bacc(None, target_bir_lowering=False, debug=False)

    x_d = nc.dram_tensor("x", [BS, C, N], XDT, kind="ExternalInput")
    vw_d = nc.dram_tensor("Vw", [R, C], F32, kind="ExternalInput")
    vb_d = nc.dram_tensor("Vb", [R, 1], F32, kind="ExternalInput")
    u_d = nc.dram_tensor("U", [M, R], F32, kind="ExternalInput")
    st_d = nc.dram_tensor("ST", [BS, N, M], F32, kind="ExternalOutput")
    tt_d = nc.dram_tensor("TT", [BS, M, C], F32, kind="ExternalOutput")

    with tile.TileContext(nc) as tc:
        with tc.tile_pool(name="const", bufs=1) as const:
            ident = const.tile([128, 128], F32)
            make_identity(nc, ident)
            ident_r = const.tile([128, 128], TRT)
            nc.scalar.copy(ident_r, ident)

            vw_sb = const.tile([R, C], F32)
            nc.sync.dma_start(vw_sb, vw_d[:])
            vb_sb = const.tile([R, 1], F32)
            nc.sync.dma_start(vb_sb, vb_d[:])
            u_sb = const.tile([M, R], F32)
            nc.sync.dma_start(u_sb, u_d[:])

            ut_sb = const.tile([R, M], F32)
            wt_sb = const.tile([128, CS, M], MMT)   # W^T tiles: [c%128, c//128, m]
            lb_sb = const.tile([M, 1], F32)

            with tc.tile_pool(name="psum_setup", bufs=1, space="PSUM") as pset:
                ut_ps = pset.tile([R, M], F32)
                nc.tensor.transpose(ut_ps, u_sb, ident[:M, :M])
                nc.vector.tensor_copy(ut_sb, ut_ps)

                wt_ps = pset.tile([128, CS, M], F32)
                for cs in range(CS):
                    nc.tensor.matmul(
                        wt_ps[:, cs, :],
                        lhsT=vw_sb[:, cs * 128:(cs + 1) * 128],
                        rhs=ut_sb,
                        start=True, stop=True,
                    )
                nc.vector.tensor_copy(wt_sb, wt_ps)

                lb_ps = pset.tile([M, 1], F32)
                nc.tensor.matmul(lb_ps, lhsT=ut_sb, rhs=vb_sb, start=True, stop=True)
                nc.vector.tensor_copy(lb_sb, lb_ps)

            with (
                tc.tile_pool(name="xpool", bufs=3) as xpool,
                tc.tile_pool(name="epool", bufs=3) as epool,
                tc.tile_pool(name="stpool", bufs=3) as stpool,
                tc.tile_pool(name="eltpool", bufs=3) as eltpool,
                tc.tile_pool(name="xtpool", bufs=4) as xtpool,
                tc.tile_pool(name="spool", bufs=4) as spool,
                tc.tile_pool(name="ttpool", bufs=2) as ttpool,
                tc.tile_pool(name="psum_l", bufs=2, space="PSUM") as psum_l,
                tc.tile_pool(name="psum_et", bufs=2, space="PSUM") as psum_et,
                tc.tile_pool(name="psum_xt", bufs=2, space="PSUM") as psum_xt,
                tc.tile_pool(name="psum_tt", bufs=2, space="PSUM") as psum_tt,
            ):
                for b in range(BS):
                    x_b = x_d[b].rearrange("(cs p) n -> p cs n", p=128)
                    st_b = st_d[b].rearrange("(ch ns p) m -> ch p ns m", ns=NS, p=128)
                    tt_ps = psum_tt.tile([M, C], F32)

                    for ch in range(CH):
                        x_sb = xpool.tile([128, CS, 512], XDT)
                        nc.sync.dma_start(x_sb, x_b[:, :, ch * 512:(ch + 1) * 512])

                        # L chunk [M, 512]
                        l_ps = psum_l.tile([M, 512], F32)
                        for cs in range(CS):
                            nc.tensor.matmul(
                                l_ps,
                                lhsT=wt_sb[:, cs, :],
                                rhs=x_sb[:, cs, :].bitcast(MMT),
                                start=(cs == 0), stop=(cs == CS - 1),
                                skip_group_check=True,
                            )

                        # expL = exp(L + Lb) on ACT, PSUM -> SBUF
                        expl_sb = epool.tile([M, 512], F32)
                        nc.scalar.activation(
                            expl_sb, l_ps, mybir.ActivationFunctionType.Exp,
                            bias=lb_sb, scale=1.0,
                        )

                        # transpose expL 128-col slices -> [n-part, m] (plain f32)
                        et_ps = psum_et.tile([128, NS, M], F32)
                        for ns in range(NS):
                            nc.tensor.transpose(
                                et_ps[:, ns, :],
                                expl_sb[:, ns * 128:(ns + 1) * 128],
                                ident[:M, :M],
                            )

                        # softmax denominator + normalize (DVE); full-precision S
                        sums = spool.tile([128, NS], F32)
                        nc.vector.reduce_sum(sums, et_ps, axis=mybir.AxisListType.X)
                        recip = spool.tile([128, NS], F32)
                        nc.vector.reciprocal(recip, sums)
                        st_sb = stpool.tile([128, NS, M], F32)
                        nc.vector.tensor_tensor(
                            st_sb, et_ps,
                            recip[:, :, None].to_broadcast((128, NS, M)),
                            mybir.AluOpType.mult,
                        )
                        nc.sync.dma_start(st_b[ch], st_sb)

                        # rounded copy of S^T for the f32r pooling matmul
                        elt_sb = eltpool.tile([128, NS, M], MMT)
                        nc.scalar.copy(elt_sb, st_sb)

                        # x^T tiles + pooling matmul
                        for ns in range(NS):
                            xt_ps = psum_xt.tile([128, C], TRT)
                            for cs in range(CS):
                                nc.tensor.transpose(
                                    xt_ps[:, cs * 128:(cs + 1) * 128],
                                    x_sb[:, cs, ns * 128:(ns + 1) * 128].bitcast(TRT),
                                    ident_r,
                                )
                            xt_sb = xtpool.tile([128, C], MMT)
                            # split PSUM->SBUF copies across ACT and DVE
                            if ns % 2 == 0:
                                nc.scalar.copy(xt_sb, xt_ps.bitcast(MMT))
                            else:
                                nc.vector.tensor_copy(xt_sb, xt_ps.bitcast(MMT))

                            nc.tensor.matmul(
                                tt_ps,
                                lhsT=elt_sb[:, ns, :],
                                rhs=xt_sb,
                                start=(ch == 0 and ns == 0),
                                stop=(ch == CH - 1 and ns == NS - 1),
                                skip_group_check=True,
                            )

                    tt_sb = ttpool.tile([M, C], F32)
                    nc.scalar.copy(tt_sb, tt_ps)
                    nc.sync.dma_start(tt_d[b], tt_sb)

    nc.compile()
    return nc


_NC = None


def _get_nc():
    global _NC
    if _NC is None:
        _NC = build_program()
    return _NC


def run(x, Vw, Vb, U, trace=False, trace_kwargs=None):
    x = np.ascontiguousarray(np.asarray(x, dtype=np.float32)).reshape(B, C, N)
    Vw = np.ascontiguousarray(np.asarray(Vw, dtype=np.float32))
    Vb = np.ascontiguousarray(np.asarray(Vb, dtype=np.float32)).reshape(R, 1)
    U = np.ascontiguousarray(np.asarray(U, dtype=np.float32))

    shards = x.reshape(NCORES, BS, C, N)
    in_maps = [
        {"x": np.ascontiguousarray(shards[i]), "Vw": Vw, "Vb": Vb, "U": U}
        for i in range(NCORES)
    ]
    nc = _get_nc()
    kw = {}
    if trace:
        kw["trace"] = True
        if trace_kwargs:
            kw["trace_kwargs"] = trace_kwargs
    res = bass_utils.run_bass_kernel_spmd(nc, in_maps, core_ids=list(range(NCORES)), **kw)
    tt = np.concatenate([r["TT"] for r in res.results], axis=0)  # [B, M, C]
    st = np.concatenate([r["ST"] for r in res.results], axis=0)  # [B, N, M]
    T = np.ascontiguousarray(tt.transpose(0, 2, 1))              # [B, C, M]
    S = np.ascontiguousarray(st.transpose(0, 2, 1))              # [B, M, N]
    return (T, S), res


def kernel(x, Vw, Vb, U):
    (T, S), _ = run(x, Vw, Vb, U, trace=False)
    return (T, S)


# ---------------------------------------------------------------------------
# Benchmarking helpers (not used by the grading path)
# ---------------------------------------------------------------------------

def _make_exec(nc, n_cores):
    """Mirror bass2jax.run_bass_via_pjrt's multi-core path, but return a jitted
    callable taking (inputs..., out_scratch...) with device-resident arrays, so
    repeated launches measure only kernel execution."""
    import jax
    from jax.experimental.shard_map import shard_map
    from jax.sharding import Mesh, NamedSharding, PartitionSpec
    from concourse import bass2jax

    bass2jax.install_neuronx_cc_hook()
    partition_name = nc.partition_id_tensor.name if nc.partition_id_tensor else None
    in_names, out_names, out_avals = [], [], []
    for alloc in nc.m.functions[0].allocations:
        if not isinstance(alloc, mybir.MemoryLocationSet):
            continue
        name = alloc.memorylocations[0].name
        if alloc.kind == "ExternalInput":
            if name != partition_name:
                in_names.append(name)
        elif alloc.kind == "ExternalOutput":
            out_names.append(name)
            out_avals.append(
                jax.core.ShapedArray(tuple(alloc.tensor_shape), mybir.dt.np(alloc.dtype))
            )
    n_params = len(in_names)
    n_outs = len(out_names)
    all_in = list(in_names) + list(out_names)
    if partition_name is not None:
        all_in.append(partition_name)

    def _body(*args):
        operands = list(args)
        if partition_name is not None:
            operands.append(bass2jax.partition_id_tensor())
        outs = bass2jax._bass_exec_p.bind(
            *operands,
            out_avals=tuple(out_avals),
            in_names=tuple(all_in),
            out_names=tuple(out_names),
            lowering_input_output_aliases=(),
            sim_require_finite=True,
            sim_require_nnan=True,
            nc=nc,
        )
        return tuple(outs)

    devices = jax.devices()[:n_cores]
    mesh = Mesh(np.asarray(devices), ("core",))
    spec = PartitionSpec("core")
    fn = jax.jit(
        shard_map(
            _body, mesh=mesh,
            in_specs=(spec,) * (n_params + n_outs),
            out_specs=(spec,) * n_outs,
            check_rep=False,
        ),
        donate_argnums=tuple(range(n_params, n_params + n_outs)),
        keep_unused=True,
    )
    sharding = NamedSharding(mesh, spec)
    return fn, in_names, out_names, out_avals, sharding


def bench(x, Vw, Vb, U, iters=20, warmup=3):
    """Measure steady-state per-launch wall time with device-resident inputs.
    Outputs of launch k are re-donated as scratch for launch k+1."""
    import time
    import jax

    x = np.ascontiguousarray(np.asarray(x, dtype=np.float32)).reshape(B, C, N)
    Vw = np.ascontiguousarray(np.asarray(Vw, dtype=np.float32))
    Vb = np.ascontiguousarray(np.asarray(Vb, dtype=np.float32)).reshape(R, 1)
    U = np.ascontiguousarray(np.asarray(U, dtype=np.float32))
    shards = x.reshape(NCORES, BS, C, N)

    nc = _get_nc()
    fn, in_names, out_names, out_avals, sharding = _make_exec(nc, NCORES)

    per_core = {
        "x": shards.reshape(NCORES * BS, C, N),
        "Vw": np.concatenate([Vw] * NCORES, 0),
        "Vb": np.concatenate([Vb] * NCORES, 0),
        "U": np.concatenate([U] * NCORES, 0),
    }
    in_dev = [jax.device_put(per_core[n], sharding) for n in in_names]
    zeros = [
        jax.device_put(np.zeros((NCORES * a.shape[0], *a.shape[1:]), a.dtype), sharding)
        for a in out_avals
    ]

    outs = fn(*in_dev, *list(zeros))
    jax.block_until_ready(outs)
    for _ in range(warmup - 1):
        outs = fn(*in_dev, *outs)
        jax.block_until_ready(outs)

    # pipelined timing
    t0 = time.perf_counter()
    for _ in range(iters):
        outs = fn(*in_dev, *outs)
    jax.block_until_ready(outs)
    t1 = time.perf_counter()
    pipelined_ns = (t1 - t0) / iters * 1e9

    # serial timing (per-launch incl round trip)
    t0 = time.perf_counter()
    for _ in range(iters):
        outs = fn(*in_dev, *outs)
        jax.block_until_ready(outs)
    t1 = time.perf_counter()
    serial_ns = (t1 - t0) / iters * 1e9

    return {"pipelined_ns": pipelined_ns, "serial_ns": serial_ns}


# revision 13
# speedup vs baseline: 34.9829x; 34.9829x over previous
"""GraphTokenPool kernel for Trainium2 (Bass/Tile), data-parallel over batch on 8 cores.

Math (per batch b):
    L = (U @ Vw) @ x + (U @ Vb)        # fused projection, [M, N]
    S = softmax(L, axis=M)             # [M, N]
    T = x @ S^T                        # [C, M]
Returns (T, S) like the reference.

Device layout choices:
  - W^T = (U@Vw)^T [C, M] computed once on device (tiny).
  - L computed in [M, N] orientation (bias per-partition works, big matmul free dim).
  - exp(L) transposed on PE (128-col slices) -> expL^T with N on partitions, so the
    softmax denominator is a free-dim reduce and S^T chunks feed the pooling matmul.
  - x chunks transposed on PE (fp32r transpose-mode) -> x^T tiles for pooling.
  - T accumulated as T^T [M, C] in a single PSUM bank across all N chunks.
  - Device outputs are S^T [N, M] and T^T [M, C]; host transposes (cheap views).
  - Matmuls run in float32r (fast PE path); the S output path stays full fp32.
"""

import numpy as np

import concourse.bass as bass
import concourse.bacc as bacc
import concourse.mybir as mybir
import concourse.tile as tile
import concourse.bass_utils as bass_utils
from concourse.masks import make_identity

F32 = mybir.dt.float32
F32R = mybir.dt.float32r

# Problem shapes (hardcoded per task contract)
B, C, H, W = 32, 384, 64, 64
N = H * W            # 4096
M, R = 64, 16
NCORES = 8
BS = B // NCORES     # 4 batches per core
CS = C // 128        # 3 c-slices
CH = N // 512        # 8 n-chunks of 512
NS = 4               # 4 sub-chunks of 128 per chunk

USE_F32R_MM = True   # float32r for the projection + pooling matmuls
USE_F32R_TR = True   # float32r for the x transposes

MMT = F32R if USE_F32R_MM else F32
TRT = F32R if USE_F32R_TR else F32
# x feeds both the f32r proj matmul and the f32r transposes
XDT = F32R if (USE_F32R_MM or USE_F32R_TR) else F32


DEFAULT_CFG = dict(
    xbufs=6, ebufs=4, stbufs=4, eltbufs=4, xtbufs=6, sbufs=8, ttbufs=2,
    ps_l=2, ps_et=2, ps_xt=3, ps_tt=1,
    st_blocked=True,   # ST DRAM layout [BS, CH, 128, NS, M] for 1KB-contiguous DMA
    # ablation flags (sim diagnostics only — break correctness)
    no_st_dma=False, no_pool=False, no_xt=False, no_proj_chain=False,
    st_dma_act=True,   # issue output DMAs on the ACT HWDGE queue (overlap with x reads)
    split_x=False,     # one DMA per c-slice instead of one per chunk
    xt_fine=False,     # per-cs xt psum tiles + copies (finer overlap)
    xt_copy="alt",     # "alt" | "act" | "dve"
    pool_pack=False,   # col-tile pooling MMs (rejected by walrus verifier; keep off)
)


def build_program(cfg=None):
    cfg = {**DEFAULT_CFG, **(cfg or {})}
    nc = bacc.Bacc(None, target_bir_lowering=False, debug=False)

    x_d = nc.dram_tensor("x", [BS, C, N], XDT, kind="ExternalInput")
    vw_d = nc.dram_tensor("Vw", [R, C], F32, kind="ExternalInput")
    vb_d = nc.dram_tensor("Vb", [R, 1], F32, kind="ExternalInput")
    u_d = nc.dram_tensor("U", [M, R], F32, kind="ExternalInput")
    if cfg["st_blocked"]:
        st_d = nc.dram_tensor("ST", [BS, CH, 128, NS, M], F32, kind="ExternalOutput")
    else:
        st_d = nc.dram_tensor("ST", [BS, N, M], F32, kind="ExternalOutput")
    tt_d = nc.dram_tensor("TT", [BS, M, C], F32, kind="ExternalOutput")

    with tile.TileContext(nc) as tc:
        with tc.tile_pool(name="const", bufs=1) as const:
            ident = const.tile([128, 128], F32)
            make_identity(nc, ident)
            ident_r = const.tile([128, 128], TRT)
            nc.scalar.copy(ident_r, ident)

            vw_sb = const.tile([R, C], F32)
            nc.sync.dma_start(vw_sb, vw_d[:])
            vb_sb = const.tile([R, 1], F32)
            nc.sync.dma_start(vb_sb, vb_d[:])
            u_sb = const.tile([M, R], F32)
            nc.sync.dma_start(u_sb, u_d[:])

            ut_sb = const.tile([R, M], F32)
            wt_sb = const.tile([128, CS, M], MMT)   # W^T tiles: [c%128, c//128, m]
            lb_sb = const.tile([M, 1], F32)

            with tc.tile_pool(name="psum_setup", bufs=1, space="PSUM") as pset:
                ut_ps = pset.tile([R, M], F32)
                nc.tensor.transpose(ut_ps, u_sb, ident[:M, :M])
                nc.vector.tensor_copy(ut_sb, ut_ps)

                wt_ps = pset.tile([128, CS, M], F32)
                for cs in range(CS):
                    nc.tensor.matmul(
                        wt_ps[:, cs, :],
                        lhsT=vw_sb[:, cs * 128:(cs + 1) * 128],
                        rhs=ut_sb,
                        start=True, stop=True,
                    )
                nc.vector.tensor_copy(wt_sb, wt_ps)

                lb_ps = pset.tile([M, 1], F32)
                nc.tensor.matmul(lb_ps, lhsT=ut_sb, rhs=vb_sb, start=True, stop=True)
                nc.vector.tensor_copy(lb_sb, lb_ps)

            with (
                tc.tile_pool(name="xpool", bufs=cfg["xbufs"]) as xpool,
                tc.tile_pool(name="epool", bufs=cfg["ebufs"]) as epool,
                tc.tile_pool(name="stpool", bufs=cfg["stbufs"]) as stpool,
                tc.tile_pool(name="eltpool", bufs=cfg["eltbufs"]) as eltpool,
                tc.tile_pool(name="xtpool", bufs=cfg["xtbufs"]) as xtpool,
                tc.tile_pool(name="spool", bufs=cfg["sbufs"]) as spool,
                tc.tile_pool(name="ttpool", bufs=cfg["ttbufs"]) as ttpool,
                tc.tile_pool(name="psum_l", bufs=cfg["ps_l"], space="PSUM") as psum_l,
                tc.tile_pool(name="psum_et", bufs=cfg["ps_et"], space="PSUM") as psum_et,
                tc.tile_pool(name="psum_xt", bufs=cfg["ps_xt"], space="PSUM") as psum_xt,
                tc.tile_pool(name="psum_tt", bufs=cfg["ps_tt"], space="PSUM") as psum_tt,
            ):
                for b in range(BS):
                    x_b = x_d[b].rearrange("(cs p) n -> p cs n", p=128)
                    if cfg["st_blocked"]:
                        st_b = st_d[b]
                    else:
                        st_b = st_d[b].rearrange("(ch ns p) m -> ch p ns m", ns=NS, p=128)
                    if cfg["pool_pack"]:
                        tt_ps = psum_tt.tile([2 * M, C], F32)
                    else:
                        tt_ps = psum_tt.tile([M, C], F32)

                    for ch in range(CH):
                        x_sb = xpool.tile([128, CS, 512], XDT)
                        if cfg["split_x"]:
                            for cs in range(CS):
                                nc.sync.dma_start(
                                    x_sb[:, cs, :],
                                    x_b[:, cs, ch * 512:(ch + 1) * 512],
                                )
                        else:
                            nc.sync.dma_start(x_sb, x_b[:, :, ch * 512:(ch + 1) * 512])

                        # L chunk [M, 512]
                        l_ps = psum_l.tile([M, 512], F32)
                        for cs in range(CS):
                            nc.tensor.matmul(
                                l_ps,
                                lhsT=wt_sb[:, cs, :],
                                rhs=x_sb[:, cs, :].bitcast(MMT),
                                start=(cs == 0), stop=(cs == CS - 1),
                                skip_group_check=True,
                            )

                        # expL = exp(L + Lb) on ACT, PSUM -> SBUF
                        expl_sb = epool.tile([M, 512], F32)
                        nc.scalar.activation(
                            expl_sb, l_ps, mybir.ActivationFunctionType.Exp,
                            bias=lb_sb, scale=1.0,
                        )

                        # transpose expL 128-col slices -> [n-part, m] (plain f32)
                        et_ps = psum_et.tile([128, NS, M], F32)
                        for ns in range(NS):
                            nc.tensor.transpose(
                                et_ps[:, ns, :],
                                expl_sb[:, ns * 128:(ns + 1) * 128],
                                ident[:M, :M],
                            )

                        # softmax denominator + normalize (DVE); full-precision S
                        sums = spool.tile([128, NS], F32)
                        nc.vector.reduce_sum(sums, et_ps, axis=mybir.AxisListType.X)
                        recip = spool.tile([128, NS], F32)
                        nc.vector.reciprocal(recip, sums)
                        st_sb = stpool.tile([128, NS, M], F32)
                        nc.vector.tensor_tensor(
                            st_sb, et_ps,
                            recip[:, :, None].to_broadcast((128, NS, M)),
                            mybir.AluOpType.mult,
                        )
                        if not cfg["no_st_dma"]:
                            st_eng = nc.scalar if cfg["st_dma_act"] else nc.sync
                            st_eng.dma_start(st_b[ch], st_sb)

                        # rounded copy of S^T for the f32r pooling matmul
                        elt_sb = eltpool.tile([128, NS, M], MMT)
                        nc.scalar.copy(elt_sb, st_sb)

                        if cfg["no_xt"]:
                            continue
                        # x^T tiles + pooling matmul
                        for ns in range(NS):
                            xt_sb = xtpool.tile([128, C], MMT)
                            if cfg["xt_fine"]:
                                for cs in range(CS):
                                    xt_ps = psum_xt.tile([128, 128], TRT, tag="xtf")
                                    nc.tensor.transpose(
                                        xt_ps,
                                        x_sb[:, cs, ns * 128:(ns + 1) * 128].bitcast(TRT),
                                        ident_r,
                                    )
                                    k = ns * CS + cs
                                    dst = xt_sb[:, cs * 128:(cs + 1) * 128]
                                    use_act = (cfg["xt_copy"] == "act") or (
                                        cfg["xt_copy"] == "alt" and k % 2 == 0)
                                    if use_act:
                                        nc.scalar.copy(dst, xt_ps.bitcast(MMT))
                                    else:
                                        nc.vector.tensor_copy(dst, xt_ps.bitcast(MMT))
                            else:
                                xt_ps = psum_xt.tile([128, C], TRT)
                                for cs in range(CS):
                                    nc.tensor.transpose(
                                        xt_ps[:, cs * 128:(cs + 1) * 128],
                                        x_sb[:, cs, ns * 128:(ns + 1) * 128].bitcast(TRT),
                                        ident_r,
                                    )
                                mode = cfg["xt_copy"]
                                if mode == "act":
                                    use_act = True
                                elif mode == "dve":
                                    use_act = False
                                elif mode == "1of4":
                                    use_act = (ns % 4 == 0)
                                else:
                                    use_act = (ns % 2 == 0)
                                if use_act:
                                    nc.scalar.copy(xt_sb, xt_ps.bitcast(MMT))
                                else:
                                    nc.vector.tensor_copy(xt_sb, xt_ps.bitcast(MMT))

                            if not cfg["no_pool"]:
                                if cfg["pool_pack"]:
                                    half = ns % 2
                                    nc.tensor.matmul(
                                        tt_ps[half * M:(half + 1) * M, :],
                                        lhsT=elt_sb[:, ns, :],
                                        rhs=xt_sb,
                                        start=(ch == 0 and ns == half),
                                        stop=(ch == CH - 1 and ns == NS - 2 + half),
                                        skip_group_check=True,
                                        tile_position=(0, half * M),
                                    )
                                else:
                                    nc.tensor.matmul(
                                        tt_ps,
                                        lhsT=elt_sb[:, ns, :],
                                        rhs=xt_sb,
                                        start=(ch == 0 and ns == 0),
                                        stop=(ch == CH - 1 and ns == NS - 1),
                                        skip_group_check=True,
                                    )

                    tt_sb = ttpool.tile([M, C], F32)
                    if cfg["pool_pack"]:
                        nc.vector.tensor_tensor(
                            tt_sb, tt_ps[:M, :], tt_ps[M:, :], mybir.AluOpType.add)
                    else:
                        nc.scalar.copy(tt_sb, tt_ps)
                    (nc.scalar if cfg["st_dma_act"] else nc.sync).dma_start(tt_d[b], tt_sb)

    nc.compile()
    return nc


_NC = None


def _get_nc():
    global _NC
    if _NC is None:
        _NC = build_program()
    return _NC


def run(x, Vw, Vb, U, trace=False, trace_kwargs=None):
    x = np.ascontiguousarray(np.asarray(x, dtype=np.float32)).reshape(B, C, N)
    Vw = np.ascontiguousarray(np.asarray(Vw, dtype=np.float32))
    Vb = np.ascontiguousarray(np.asarray(Vb, dtype=np.float32)).reshape(R, 1)
    U = np.ascontiguousarray(np.asarray(U, dtype=np.float32))

    shards = x.reshape(NCORES, BS, C, N)
    in_maps = [
        {"x": np.ascontiguousarray(shards[i]), "Vw": Vw, "Vb": Vb, "U": U}
        for i in range(NCORES)
    ]
    nc = _get_nc()
    kw = {}
    if trace:
        kw["trace"] = True
        if trace_kwargs:
            kw["trace_kwargs"] = trace_kwargs
    res = bass_utils.run_bass_kernel_spmd(nc, in_maps, core_ids=list(range(NCORES)), **kw)
    tt = np.concatenate([r["TT"] for r in res.results], axis=0)  # [B, M, C]
    T = np.ascontiguousarray(tt.transpose(0, 2, 1))              # [B, C, M]
    st = np.concatenate([r["ST"] for r in res.results], axis=0)
    if DEFAULT_CFG["st_blocked"]:
        # st: [B, CH, 128, NS, M]; n = ch*512 + ns*128 + p
        S = np.ascontiguousarray(
            st.transpose(0, 4, 1, 3, 2).reshape(B, M, N))        # [B, M, N]
    else:
        S = np.ascontiguousarray(st.transpose(0, 2, 1))          # [B, M, N]
    return (T, S), res


def kernel(x, Vw, Vb, U):
    (T, S), _ = run(x, Vw, Vb, U, trace=False)
    return (T, S)


# ---------------------------------------------------------------------------
# Benchmarking helpers (not used by the grading path)
# ---------------------------------------------------------------------------

def _make_exec(nc, n_cores):
    """Mirror bass2jax.run_bass_via_pjrt's multi-core path, but return a jitted
    callable taking (inputs..., out_scratch...) with device-resident arrays, so
    repeated launches measure only kernel execution."""
    import jax
    from jax.experimental.shard_map import shard_map
    from jax.sharding import Mesh, NamedSharding, PartitionSpec
    from concourse import bass2jax

    bass2jax.install_neuronx_cc_hook()
    partition_name = nc.partition_id_tensor.name if nc.partition_id_tensor else None
    in_names, out_names, out_avals = [], [], []
    for alloc in nc.m.functions[0].allocations:
        if not isinstance(alloc, mybir.MemoryLocationSet):
            continue
        name = alloc.memorylocations[0].name
        if alloc.kind == "ExternalInput":
            if name != partition_name:
                in_names.append(name)
        elif alloc.kind == "ExternalOutput":
            out_names.append(name)
            out_avals.append(
                jax.core.ShapedArray(tuple(alloc.tensor_shape), mybir.dt.np(alloc.dtype))
            )
    n_params = len(in_names)
    n_outs = len(out_names)
    all_in = list(in_names) + list(out_names)
    if partition_name is not None:
        all_in.append(partition_name)

    def _body(*args):
        operands = list(args)
        if partition_name is not None:
            operands.append(bass2jax.partition_id_tensor())
        outs = bass2jax._bass_exec_p.bind(
            *operands,
            out_avals=tuple(out_avals),
            in_names=tuple(all_in),
            out_names=tuple(out_names),
            lowering_input_output_aliases=(),
            sim_require_finite=True,
            sim_require_nnan=True,
            nc=nc,
        )
        return tuple(outs)

    devices = jax.devices()[:n_cores]
    mesh = Mesh(np.asarray(devices), ("core",))
    spec = PartitionSpec("core")
    fn = jax.jit(
        shard_map(
            _body, mesh=mesh,
            in_specs=(spec,) * (n_params + n_outs),
            out_specs=(spec,) * n_outs,
            check_rep=False,
        ),
        donate_argnums=tuple(range(n_params, n_params + n_outs)),
        keep_unused=True,
    )
    sharding = NamedSharding(mesh, spec)
    return fn, in_names, out_names, out_avals, sharding


def bench(x, Vw, Vb, U, iters=20, warmup=3):
    """Measure steady-state per-launch wall time with device-resident inputs.
    Outputs of launch k are re-donated as scratch for launch k+1."""
    import time
    import jax

    x = np.ascontiguousarray(np.asarray(x, dtype=np.float32)).reshape(B, C, N)
    Vw = np.ascontiguousarray(np.asarray(Vw, dtype=np.float32))
    Vb = np.ascontiguousarray(np.asarray(Vb, dtype=np.float32)).reshape(R, 1)
    U = np.ascontiguousarray(np.asarray(U, dtype=np.float32))
    shards = x.reshape(NCORES, BS, C, N)

    nc = _get_nc()
    fn, in_names, out_names, out_avals, sharding = _make_exec(nc, NCORES)

    per_core = {
        "x": shards.reshape(NCORES * BS, C, N),
        "Vw": np.concatenate([Vw] * NCORES, 0),
        "Vb": np.concatenate([Vb] * NCORES, 0),
        "U": np.concatenate([U] * NCORES, 0),
    }
    in_dev = [jax.device_put(per_core[n], sharding) for n in in_names]
    zeros = [
        jax.device_put(np.zeros((NCORES * a.shape[0], *a.shape[1:]), a.dtype), sharding)
        for a in out_avals
    ]

    outs = fn(*in_dev, *list(zeros))
    jax.block_until_ready(outs)
    for _ in range(warmup - 1):
        outs = fn(*in_dev, *outs)
        jax.block_until_ready(outs)

    # pipelined timing
    t0 = time.perf_counter()
    for _ in range(iters):
        outs = fn(*in_dev, *outs)
    jax.block_until_ready(outs)
    t1 = time.perf_counter()
    pipelined_ns = (t1 - t0) / iters * 1e9

    # serial timing (per-launch incl round trip)
    t0 = time.perf_counter()
    for _ in range(iters):
        outs = fn(*in_dev, *outs)
        jax.block_until_ready(outs)
    t1 = time.perf_counter()
    serial_ns = (t1 - t0) / iters * 1e9

    return {"pipelined_ns": pipelined_ns, "serial_ns": serial_ns}


# revision 14
# speedup vs baseline: 36.4687x; 1.0425x over previous
"""GraphTokenPool kernel for Trainium2 (Bass/Tile), data-parallel over batch on 8 cores.

Math (per batch b):
    L = (U @ Vw) @ x + (U @ Vb)        # fused projection, [M, N]
    S = softmax(L, axis=M)             # [M, N]
    T = x @ S^T                        # [C, M]
Returns (T, S) like the reference.

Device layout choices:
  - W^T = (U@Vw)^T [C, M] computed once on device (tiny).
  - L computed in [M, N] orientation (bias per-partition works, big matmul free dim).
  - exp(L) transposed on PE (128-col slices) -> expL^T with N on partitions, so the
    softmax denominator is a free-dim reduce and S^T chunks feed the pooling matmul.
  - x chunks transposed on PE (fp32r transpose-mode) -> x^T tiles for pooling.
  - T accumulated as T^T [M, C] in a single PSUM bank across all N chunks.
  - Device outputs are S^T [N, M] and T^T [M, C]; host transposes (cheap views).
  - Matmuls run in float32r (fast PE path); the S output path stays full fp32.
"""

import numpy as np

import concourse.bass as bass
import concourse.bacc as bacc
import concourse.mybir as mybir
import concourse.tile as tile
import concourse.bass_utils as bass_utils
from concourse.masks import make_identity

F32 = mybir.dt.float32
F32R = mybir.dt.float32r

# Problem shapes (hardcoded per task contract)
B, C, H, W = 32, 384, 64, 64
N = H * W            # 4096
M, R = 64, 16
NCORES = 8
BS = B // NCORES     # 4 batches per core
CS = C // 128        # 3 c-slices
CH = N // 512        # 8 n-chunks of 512
NS = 4               # 4 sub-chunks of 128 per chunk

USE_F32R_MM = True   # float32r for the projection + pooling matmuls
USE_F32R_TR = True   # float32r for the x transposes

MMT = F32R if USE_F32R_MM else F32
TRT = F32R if USE_F32R_TR else F32
# x feeds both the f32r proj matmul and the f32r transposes
XDT = F32R if (USE_F32R_MM or USE_F32R_TR) else F32


DEFAULT_CFG = dict(
    xbufs=6, ebufs=4, stbufs=4, eltbufs=4, xtbufs=6, sbufs=8, ttbufs=2,
    ps_l=2, ps_et=2, ps_xt=3, ps_tt=1,
    st_blocked=True,   # ST DRAM layout [BS, CH, 128, NS, M] for 1KB-contiguous DMA
    # ablation flags (sim diagnostics only — break correctness)
    no_st_dma=False, no_pool=False, no_xt=False, no_proj_chain=False,
    st_dma_act=True,   # issue output DMAs on the ACT HWDGE queue (overlap with x reads)
    split_x=False,     # one DMA per c-slice instead of one per chunk
    xt_fine=False,     # per-cs xt psum tiles + copies (finer overlap)
    xt_copy="alt",     # "alt" | "act" | "dve"
    pool_pack=False,   # col-tile pooling MMs (rejected by walrus verifier; keep off)
    skip_elt=True,     # produce S^T as f32r on DVE; pool reads it directly (no ACT copy)
)


def build_program(cfg=None):
    cfg = {**DEFAULT_CFG, **(cfg or {})}
    nc = bacc.Bacc(None, target_bir_lowering=False, debug=False)

    x_d = nc.dram_tensor("x", [BS, C, N], XDT, kind="ExternalInput")
    vw_d = nc.dram_tensor("Vw", [R, C], F32, kind="ExternalInput")
    vb_d = nc.dram_tensor("Vb", [R, 1], F32, kind="ExternalInput")
    u_d = nc.dram_tensor("U", [M, R], F32, kind="ExternalInput")
    if cfg["st_blocked"]:
        st_d = nc.dram_tensor("ST", [BS, CH, 128, NS, M], F32, kind="ExternalOutput")
    else:
        st_d = nc.dram_tensor("ST", [BS, N, M], F32, kind="ExternalOutput")
    tt_d = nc.dram_tensor("TT", [BS, M, C], F32, kind="ExternalOutput")

    with tile.TileContext(nc) as tc:
        with tc.tile_pool(name="const", bufs=1) as const:
            ident = const.tile([128, 128], F32)
            make_identity(nc, ident)
            ident_r = const.tile([128, 128], TRT)
            nc.scalar.copy(ident_r, ident)

            vw_sb = const.tile([R, C], F32)
            nc.sync.dma_start(vw_sb, vw_d[:])
            vb_sb = const.tile([R, 1], F32)
            nc.sync.dma_start(vb_sb, vb_d[:])
            u_sb = const.tile([M, R], F32)
            nc.sync.dma_start(u_sb, u_d[:])

            ut_sb = const.tile([R, M], F32)
            wt_sb = const.tile([128, CS, M], MMT)   # W^T tiles: [c%128, c//128, m]
            lb_sb = const.tile([M, 1], F32)

            with tc.tile_pool(name="psum_setup", bufs=1, space="PSUM") as pset:
                ut_ps = pset.tile([R, M], F32)
                nc.tensor.transpose(ut_ps, u_sb, ident[:M, :M])
                nc.vector.tensor_copy(ut_sb, ut_ps)

                wt_ps = pset.tile([128, CS, M], F32)
                for cs in range(CS):
                    nc.tensor.matmul(
                        wt_ps[:, cs, :],
                        lhsT=vw_sb[:, cs * 128:(cs + 1) * 128],
                        rhs=ut_sb,
                        start=True, stop=True,
                    )
                nc.vector.tensor_copy(wt_sb, wt_ps)

                lb_ps = pset.tile([M, 1], F32)
                nc.tensor.matmul(lb_ps, lhsT=ut_sb, rhs=vb_sb, start=True, stop=True)
                nc.vector.tensor_copy(lb_sb, lb_ps)

            with (
                tc.tile_pool(name="xpool", bufs=cfg["xbufs"]) as xpool,
                tc.tile_pool(name="epool", bufs=cfg["ebufs"]) as epool,
                tc.tile_pool(name="stpool", bufs=cfg["stbufs"]) as stpool,
                tc.tile_pool(name="eltpool", bufs=cfg["eltbufs"]) as eltpool,
                tc.tile_pool(name="xtpool", bufs=cfg["xtbufs"]) as xtpool,
                tc.tile_pool(name="spool", bufs=cfg["sbufs"]) as spool,
                tc.tile_pool(name="ttpool", bufs=cfg["ttbufs"]) as ttpool,
                tc.tile_pool(name="psum_l", bufs=cfg["ps_l"], space="PSUM") as psum_l,
                tc.tile_pool(name="psum_et", bufs=cfg["ps_et"], space="PSUM") as psum_et,
                tc.tile_pool(name="psum_xt", bufs=cfg["ps_xt"], space="PSUM") as psum_xt,
                tc.tile_pool(name="psum_tt", bufs=cfg["ps_tt"], space="PSUM") as psum_tt,
            ):
                for b in range(BS):
                    x_b = x_d[b].rearrange("(cs p) n -> p cs n", p=128)
                    if cfg["st_blocked"]:
                        st_b = st_d[b]
                    else:
                        st_b = st_d[b].rearrange("(ch ns p) m -> ch p ns m", ns=NS, p=128)
                    if cfg["pool_pack"]:
                        tt_ps = psum_tt.tile([2 * M, C], F32)
                    else:
                        tt_ps = psum_tt.tile([M, C], F32)

                    for ch in range(CH):
                        x_sb = xpool.tile([128, CS, 512], XDT)
                        if cfg["split_x"]:
                            for cs in range(CS):
                                nc.sync.dma_start(
                                    x_sb[:, cs, :],
                                    x_b[:, cs, ch * 512:(ch + 1) * 512],
                                )
                        else:
                            nc.sync.dma_start(x_sb, x_b[:, :, ch * 512:(ch + 1) * 512])

                        # L chunk [M, 512]
                        l_ps = psum_l.tile([M, 512], F32)
                        for cs in range(CS):
                            nc.tensor.matmul(
                                l_ps,
                                lhsT=wt_sb[:, cs, :],
                                rhs=x_sb[:, cs, :].bitcast(MMT),
                                start=(cs == 0), stop=(cs == CS - 1),
                                skip_group_check=True,
                            )

                        # expL = exp(L + Lb) on ACT, PSUM -> SBUF
                        expl_sb = epool.tile([M, 512], F32)
                        nc.scalar.activation(
                            expl_sb, l_ps, mybir.ActivationFunctionType.Exp,
                            bias=lb_sb, scale=1.0,
                        )

                        # transpose expL 128-col slices -> [n-part, m] (plain f32)
                        et_ps = psum_et.tile([128, NS, M], F32)
                        for ns in range(NS):
                            nc.tensor.transpose(
                                et_ps[:, ns, :],
                                expl_sb[:, ns * 128:(ns + 1) * 128],
                                ident[:M, :M],
                            )

                        # softmax denominator + normalize (DVE); full-precision S
                        sums = spool.tile([128, NS], F32)
                        nc.vector.reduce_sum(sums, et_ps, axis=mybir.AxisListType.X)
                        recip = spool.tile([128, NS], F32)
                        nc.vector.reciprocal(recip, sums)
                        if cfg["skip_elt"]:
                            st_sb = stpool.tile([128, NS, M], MMT)
                        else:
                            st_sb = stpool.tile([128, NS, M], F32)
                        nc.vector.tensor_tensor(
                            st_sb, et_ps,
                            recip[:, :, None].to_broadcast((128, NS, M)),
                            mybir.AluOpType.mult,
                        )
                        if not cfg["no_st_dma"]:
                            st_eng = nc.scalar if cfg["st_dma_act"] else nc.sync
                            st_eng.dma_start(st_b[ch], st_sb.bitcast(F32)
                                             if cfg["skip_elt"] else st_sb)

                        if cfg["skip_elt"]:
                            elt_sb = st_sb
                        else:
                            # rounded copy of S^T for the f32r pooling matmul
                            elt_sb = eltpool.tile([128, NS, M], MMT)
                            nc.scalar.copy(elt_sb, st_sb)

                        if cfg["no_xt"]:
                            continue
                        # x^T tiles + pooling matmul
                        for ns in range(NS):
                            xt_sb = xtpool.tile([128, C], MMT)
                            if cfg["xt_fine"]:
                                for cs in range(CS):
                                    xt_ps = psum_xt.tile([128, 128], TRT, tag="xtf")
                                    nc.tensor.transpose(
                                        xt_ps,
                                        x_sb[:, cs, ns * 128:(ns + 1) * 128].bitcast(TRT),
                                        ident_r,
                                    )
                                    k = ns * CS + cs
                                    dst = xt_sb[:, cs * 128:(cs + 1) * 128]
                                    use_act = (cfg["xt_copy"] == "act") or (
                                        cfg["xt_copy"] == "alt" and k % 2 == 0)
                                    if use_act:
                                        nc.scalar.copy(dst, xt_ps.bitcast(MMT))
                                    else:
                                        nc.vector.tensor_copy(dst, xt_ps.bitcast(MMT))
                            else:
                                xt_ps = psum_xt.tile([128, C], TRT)
                                for cs in range(CS):
                                    nc.tensor.transpose(
                                        xt_ps[:, cs * 128:(cs + 1) * 128],
                                        x_sb[:, cs, ns * 128:(ns + 1) * 128].bitcast(TRT),
                                        ident_r,
                                    )
                                mode = cfg["xt_copy"]
                                if mode == "act":
                                    use_act = True
                                elif mode == "dve":
                                    use_act = False
                                elif mode == "1of4":
                                    use_act = (ns % 4 == 0)
                                else:
                                    use_act = (ns % 2 == 0)
                                if use_act:
                                    nc.scalar.copy(xt_sb, xt_ps.bitcast(MMT))
                                else:
                                    nc.vector.tensor_copy(xt_sb, xt_ps.bitcast(MMT))

                            if not cfg["no_pool"]:
                                if cfg["pool_pack"]:
                                    half = ns % 2
                                    nc.tensor.matmul(
                                        tt_ps[half * M:(half + 1) * M, :],
                                        lhsT=elt_sb[:, ns, :],
                                        rhs=xt_sb,
                                        start=(ch == 0 and ns == half),
                                        stop=(ch == CH - 1 and ns == NS - 2 + half),
                                        skip_group_check=True,
                                        tile_position=(0, half * M),
                                    )
                                else:
                                    nc.tensor.matmul(
                                        tt_ps,
                                        lhsT=elt_sb[:, ns, :],
                                        rhs=xt_sb,
                                        start=(ch == 0 and ns == 0),
                                        stop=(ch == CH - 1 and ns == NS - 1),
                                        skip_group_check=True,
                                    )

                    tt_sb = ttpool.tile([M, C], F32)
                    if cfg["pool_pack"]:
                        nc.vector.tensor_tensor(
                            tt_sb, tt_ps[:M, :], tt_ps[M:, :], mybir.AluOpType.add)
                    else:
                        nc.scalar.copy(tt_sb, tt_ps)
                    (nc.scalar if cfg["st_dma_act"] else nc.sync).dma_start(tt_d[b], tt_sb)

    nc.compile()
    return nc


_NC = None


def _get_nc():
    global _NC
    if _NC is None:
        _NC = build_program()
    return _NC


def run(x, Vw, Vb, U, trace=False, trace_kwargs=None):
    x = np.ascontiguousarray(np.asarray(x, dtype=np.float32)).reshape(B, C, N)
    Vw = np.ascontiguousarray(np.asarray(Vw, dtype=np.float32))
    Vb = np.ascontiguousarray(np.asarray(Vb, dtype=np.float32)).reshape(R, 1)
    U = np.ascontiguousarray(np.asarray(U, dtype=np.float32))

    shards = x.reshape(NCORES, BS, C, N)
    in_maps = [
        {"x": np.ascontiguousarray(shards[i]), "Vw": Vw, "Vb": Vb, "U": U}
        for i in range(NCORES)
    ]
    nc = _get_nc()
    kw = {}
    if trace:
        kw["trace"] = True
        if trace_kwargs:
            kw["trace_kwargs"] = trace_kwargs
    res = bass_utils.run_bass_kernel_spmd(nc, in_maps, core_ids=list(range(NCORES)), **kw)
    tt = np.concatenate([r["TT"] for r in res.results], axis=0)  # [B, M, C]
    T = np.ascontiguousarray(tt.transpose(0, 2, 1))              # [B, C, M]
    st = np.concatenate([r["ST"] for r in res.results], axis=0)
    if DEFAULT_CFG["st_blocked"]:
        # st: [B, CH, 128, NS, M]; n = ch*512 + ns*128 + p
        S = np.ascontiguousarray(
            st.transpose(0, 4, 1, 3, 2).reshape(B, M, N))        # [B, M, N]
    else:
        S = np.ascontiguousarray(st.transpose(0, 2, 1))          # [B, M, N]
    return (T, S), res


def kernel(x, Vw, Vb, U):
    (T, S), _ = run(x, Vw, Vb, U, trace=False)
    return (T, S)


# ---------------------------------------------------------------------------
# Benchmarking helpers (not used by the grading path)
# ---------------------------------------------------------------------------

def _make_exec(nc, n_cores):
    """Mirror bass2jax.run_bass_via_pjrt's multi-core path, but return a jitted
    callable taking (inputs..., out_scratch...) with device-resident arrays, so
    repeated launches measure only kernel execution."""
    import jax
    from jax.experimental.shard_map import shard_map
    from jax.sharding import Mesh, NamedSharding, PartitionSpec
    from concourse import bass2jax

    bass2jax.install_neuronx_cc_hook()
    partition_name = nc.partition_id_tensor.name if nc.partition_id_tensor else None
    in_names, out_names, out_avals = [], [], []
    for alloc in nc.m.functions[0].allocations:
        if not isinstance(alloc, mybir.MemoryLocationSet):
            continue
        name = alloc.memorylocations[0].name
        if alloc.kind == "ExternalInput":
            if name != partition_name:
                in_names.append(name)
        elif alloc.kind == "ExternalOutput":
            out_names.append(name)
            out_avals.append(
                jax.core.ShapedArray(tuple(alloc.tensor_shape), mybir.dt.np(alloc.dtype))
            )
    n_params = len(in_names)
    n_outs = len(out_names)
    all_in = list(in_names) + list(out_names)
    if partition_name is not None:
        all_in.append(partition_name)

    def _body(*args):
        operands = list(args)
        if partition_name is not None:
            operands.append(bass2jax.partition_id_tensor())
        outs = bass2jax._bass_exec_p.bind(
            *operands,
            out_avals=tuple(out_avals),
            in_names=tuple(all_in),
            out_names=tuple(out_names),
            lowering_input_output_aliases=(),
            sim_require_finite=True,
            sim_require_nnan=True,
            nc=nc,
        )
        return tuple(outs)

    devices = jax.devices()[:n_cores]
    mesh = Mesh(np.asarray(devices), ("core",))
    spec = PartitionSpec("core")
    fn = jax.jit(
        shard_map(
            _body, mesh=mesh,
            in_specs=(spec,) * (n_params + n_outs),
            out_specs=(spec,) * n_outs,
            check_rep=False,
        ),
        donate_argnums=tuple(range(n_params, n_params + n_outs)),
        keep_unused=True,
    )
    sharding = NamedSharding(mesh, spec)
    return fn, in_names, out_names, out_avals, sharding


def bench(x, Vw, Vb, U, iters=20, warmup=3):
    """Measure steady-state per-launch wall time with device-resident inputs.
    Outputs of launch k are re-donated as scratch for launch k+1."""
    import time
    import jax

    x = np.ascontiguousarray(np.asarray(x, dtype=np.float32)).reshape(B, C, N)
    Vw = np.ascontiguousarray(np.asarray(Vw, dtype=np.float32))
    Vb = np.ascontiguousarray(np.asarray(Vb, dtype=np.float32)).reshape(R, 1)
    U = np.ascontiguousarray(np.asarray(U, dtype=np.float32))
    shards = x.reshape(NCORES, BS, C, N)

    nc = _get_nc()
    fn, in_names, out_names, out_avals, sharding = _make_exec(nc, NCORES)

    per_core = {
        "x": shards.reshape(NCORES * BS, C, N),
        "Vw": np.concatenate([Vw] * NCORES, 0),
        "Vb": np.concatenate([Vb] * NCORES, 0),
        "U": np.concatenate([U] * NCORES, 0),
    }
    in_dev = [jax.device_put(per_core[n], sharding) for n in in_names]
    zeros = [
        jax.device_put(np.zeros((NCORES * a.shape[0], *a.shape[1:]), a.dtype), sharding)
        for a in out_avals
    ]

    outs = fn(*in_dev, *list(zeros))
    jax.block_until_ready(outs)
    for _ in range(warmup - 1):
        outs = fn(*in_dev, *outs)
        jax.block_until_ready(outs)

    # pipelined timing
    t0 = time.perf_counter()
    for _ in range(iters):
        outs = fn(*in_dev, *outs)
    jax.block_until_ready(outs)
    t1 = time.perf_counter()
    pipelined_ns = (t1 - t0) / iters * 1e9

    # serial timing (per-launch incl round trip)
    t0 = time.perf_counter()
    for _ in range(iters):
        outs = fn(*in_dev, *outs)
        jax.block_until_ready(outs)
    t1 = time.perf_counter()
    serial_ns = (t1 - t0) / iters * 1e9

    return {"pipelined_ns": pipelined_ns, "serial_ns": serial_ns}


# revision 15
# speedup vs baseline: 36.5996x; 1.0036x over previous
"""GraphTokenPool kernel for Trainium2 (Bass/Tile), data-parallel over batch on 8 cores.

Math (per batch b):
    L = (U @ Vw) @ x + (U @ Vb)        # fused projection, [M, N]
    S = softmax(L, axis=M)             # [M, N]
    T = x @ S^T                        # [C, M]
Returns (T, S) like the reference.

Device layout choices:
  - W^T = (U@Vw)^T [C, M] computed once on device (tiny).
  - L computed in [M, N] orientation (bias per-partition works, big matmul free dim).
  - exp(L) transposed on PE (128-col slices) -> expL^T with N on partitions, so the
    softmax denominator is a free-dim reduce and S^T chunks feed the pooling matmul.
  - x chunks transposed on PE (fp32r transpose-mode) -> x^T tiles for pooling.
  - T accumulated as T^T [M, C] in a single PSUM bank across all N chunks.
  - Device outputs are S^T [N, M] and T^T [M, C]; host transposes (cheap views).
  - Matmuls run in float32r (fast PE path); the S output path stays full fp32.
"""

import numpy as np

import concourse.bass as bass
import concourse.bacc as bacc
import concourse.mybir as mybir
import concourse.tile as tile
import concourse.bass_utils as bass_utils
from concourse.masks import make_identity

F32 = mybir.dt.float32
F32R = mybir.dt.float32r

# Problem shapes (hardcoded per task contract)
B, C, H, W = 32, 384, 64, 64
N = H * W            # 4096
M, R = 64, 16
NCORES = 8
BS = B // NCORES     # 4 batches per core
CS = C // 128        # 3 c-slices
CH = N // 512        # 8 n-chunks of 512
NS = 4               # 4 sub-chunks of 128 per chunk

USE_F32R_MM = True   # float32r for the projection + pooling matmuls
USE_F32R_TR = True   # float32r for the x transposes

MMT = F32R if USE_F32R_MM else F32
TRT = F32R if USE_F32R_TR else F32
# x feeds both the f32r proj matmul and the f32r transposes
XDT = F32R if (USE_F32R_MM or USE_F32R_TR) else F32


DEFAULT_CFG = dict(
    xbufs=6, ebufs=4, stbufs=4, eltbufs=4, xtbufs=6, sbufs=8, ttbufs=2,
    ps_l=2, ps_et=2, ps_xt=3, ps_tt=1,
    st_blocked=True,   # ST DRAM layout [BS, CH, 128, NS, M] for 1KB-contiguous DMA
    # ablation flags (sim diagnostics only — break correctness)
    no_st_dma=False, no_pool=False, no_xt=False, no_proj_chain=False,
    st_dma_act=True,   # issue output DMAs on the ACT HWDGE queue (overlap with x reads)
    split_x=False,     # one DMA per c-slice instead of one per chunk
    xt_fine=False,     # per-cs xt psum tiles + copies (finer overlap)
    xt_copy="alt",     # "alt" | "act" | "dve"
    pool_pack=False,   # col-tile pooling MMs (rejected by walrus verifier; keep off)
    skip_elt=True,     # produce S^T as f32r on DVE; pool reads it directly (no ACT copy)
    et_f32r=True,      # exp(L) written as f32r so expL transposes run at 1.5 cyc/row
)


def build_program(cfg=None):
    cfg = {**DEFAULT_CFG, **(cfg or {})}
    nc = bacc.Bacc(None, target_bir_lowering=False, debug=False)

    x_d = nc.dram_tensor("x", [BS, C, N], XDT, kind="ExternalInput")
    vw_d = nc.dram_tensor("Vw", [R, C], F32, kind="ExternalInput")
    vb_d = nc.dram_tensor("Vb", [R, 1], F32, kind="ExternalInput")
    u_d = nc.dram_tensor("U", [M, R], F32, kind="ExternalInput")
    if cfg["st_blocked"]:
        st_d = nc.dram_tensor("ST", [BS, CH, 128, NS, M], F32, kind="ExternalOutput")
    else:
        st_d = nc.dram_tensor("ST", [BS, N, M], F32, kind="ExternalOutput")
    tt_d = nc.dram_tensor("TT", [BS, M, C], F32, kind="ExternalOutput")

    with tile.TileContext(nc) as tc:
        with tc.tile_pool(name="const", bufs=1) as const:
            ident = const.tile([128, 128], F32)
            make_identity(nc, ident)
            ident_r = const.tile([128, 128], TRT)
            nc.scalar.copy(ident_r, ident)

            vw_sb = const.tile([R, C], F32)
            nc.sync.dma_start(vw_sb, vw_d[:])
            vb_sb = const.tile([R, 1], F32)
            nc.sync.dma_start(vb_sb, vb_d[:])
            u_sb = const.tile([M, R], F32)
            nc.sync.dma_start(u_sb, u_d[:])

            ut_sb = const.tile([R, M], F32)
            wt_sb = const.tile([128, CS, M], MMT)   # W^T tiles: [c%128, c//128, m]
            lb_sb = const.tile([M, 1], F32)

            with tc.tile_pool(name="psum_setup", bufs=1, space="PSUM") as pset:
                ut_ps = pset.tile([R, M], F32)
                nc.tensor.transpose(ut_ps, u_sb, ident[:M, :M])
                nc.vector.tensor_copy(ut_sb, ut_ps)

                wt_ps = pset.tile([128, CS, M], F32)
                for cs in range(CS):
                    nc.tensor.matmul(
                        wt_ps[:, cs, :],
                        lhsT=vw_sb[:, cs * 128:(cs + 1) * 128],
                        rhs=ut_sb,
                        start=True, stop=True,
                    )
                nc.vector.tensor_copy(wt_sb, wt_ps)

                lb_ps = pset.tile([M, 1], F32)
                nc.tensor.matmul(lb_ps, lhsT=ut_sb, rhs=vb_sb, start=True, stop=True)
                nc.vector.tensor_copy(lb_sb, lb_ps)

            with (
                tc.tile_pool(name="xpool", bufs=cfg["xbufs"]) as xpool,
                tc.tile_pool(name="epool", bufs=cfg["ebufs"]) as epool,
                tc.tile_pool(name="stpool", bufs=cfg["stbufs"]) as stpool,
                tc.tile_pool(name="eltpool", bufs=cfg["eltbufs"]) as eltpool,
                tc.tile_pool(name="xtpool", bufs=cfg["xtbufs"]) as xtpool,
                tc.tile_pool(name="spool", bufs=cfg["sbufs"]) as spool,
                tc.tile_pool(name="ttpool", bufs=cfg["ttbufs"]) as ttpool,
                tc.tile_pool(name="psum_l", bufs=cfg["ps_l"], space="PSUM") as psum_l,
                tc.tile_pool(name="psum_et", bufs=cfg["ps_et"], space="PSUM") as psum_et,
                tc.tile_pool(name="psum_xt", bufs=cfg["ps_xt"], space="PSUM") as psum_xt,
                tc.tile_pool(name="psum_tt", bufs=cfg["ps_tt"], space="PSUM") as psum_tt,
            ):
                for b in range(BS):
                    x_b = x_d[b].rearrange("(cs p) n -> p cs n", p=128)
                    if cfg["st_blocked"]:
                        st_b = st_d[b]
                    else:
                        st_b = st_d[b].rearrange("(ch ns p) m -> ch p ns m", ns=NS, p=128)
                    if cfg["pool_pack"]:
                        tt_ps = psum_tt.tile([2 * M, C], F32)
                    else:
                        tt_ps = psum_tt.tile([M, C], F32)

                    for ch in range(CH):
                        x_sb = xpool.tile([128, CS, 512], XDT)
                        if cfg["split_x"]:
                            for cs in range(CS):
                                nc.sync.dma_start(
                                    x_sb[:, cs, :],
                                    x_b[:, cs, ch * 512:(ch + 1) * 512],
                                )
                        else:
                            nc.sync.dma_start(x_sb, x_b[:, :, ch * 512:(ch + 1) * 512])

                        # L chunk [M, 512]
                        l_ps = psum_l.tile([M, 512], F32)
                        for cs in range(CS):
                            nc.tensor.matmul(
                                l_ps,
                                lhsT=wt_sb[:, cs, :],
                                rhs=x_sb[:, cs, :].bitcast(MMT),
                                start=(cs == 0), stop=(cs == CS - 1),
                                skip_group_check=True,
                            )

                        # expL = exp(L + Lb) on ACT, PSUM -> SBUF
                        ET = MMT if cfg["et_f32r"] else F32
                        expl_sb = epool.tile([M, 512], ET)
                        nc.scalar.activation(
                            expl_sb, l_ps, mybir.ActivationFunctionType.Exp,
                            bias=lb_sb, scale=1.0,
                        )

                        # transpose expL 128-col slices -> [n-part, m]
                        et_ps = psum_et.tile([128, NS, M], ET)
                        id_et = ident_r if cfg["et_f32r"] else ident
                        for ns in range(NS):
                            nc.tensor.transpose(
                                et_ps[:, ns, :],
                                expl_sb[:, ns * 128:(ns + 1) * 128],
                                id_et[:M, :M],
                            )

                        # softmax denominator + normalize (DVE); full-precision S
                        sums = spool.tile([128, NS], F32)
                        nc.vector.reduce_sum(sums, et_ps.bitcast(F32),
                                             axis=mybir.AxisListType.X)
                        recip = spool.tile([128, NS], F32)
                        nc.vector.reciprocal(recip, sums)
                        if cfg["skip_elt"]:
                            st_sb = stpool.tile([128, NS, M], MMT)
                        else:
                            st_sb = stpool.tile([128, NS, M], F32)
                        nc.vector.tensor_tensor(
                            st_sb, et_ps.bitcast(F32),
                            recip[:, :, None].to_broadcast((128, NS, M)),
                            mybir.AluOpType.mult,
                        )
                        if not cfg["no_st_dma"]:
                            st_eng = nc.scalar if cfg["st_dma_act"] else nc.sync
                            st_eng.dma_start(st_b[ch], st_sb.bitcast(F32)
                                             if cfg["skip_elt"] else st_sb)

                        if cfg["skip_elt"]:
                            elt_sb = st_sb
                        else:
                            # rounded copy of S^T for the f32r pooling matmul
                            elt_sb = eltpool.tile([128, NS, M], MMT)
                            nc.scalar.copy(elt_sb, st_sb)

                        if cfg["no_xt"]:
                            continue
                        # x^T tiles + pooling matmul
                        for ns in range(NS):
                            xt_sb = xtpool.tile([128, C], MMT)
                            if cfg["xt_fine"]:
                                for cs in range(CS):
                                    xt_ps = psum_xt.tile([128, 128], TRT, tag="xtf")
                                    nc.tensor.transpose(
                                        xt_ps,
                                        x_sb[:, cs, ns * 128:(ns + 1) * 128].bitcast(TRT),
                                        ident_r,
                                    )
                                    k = ns * CS + cs
                                    dst = xt_sb[:, cs * 128:(cs + 1) * 128]
                                    use_act = (cfg["xt_copy"] == "act") or (
                                        cfg["xt_copy"] == "alt" and k % 2 == 0)
                                    if use_act:
                                        nc.scalar.copy(dst, xt_ps.bitcast(MMT))
                                    else:
                                        nc.vector.tensor_copy(dst, xt_ps.bitcast(MMT))
                            else:
                                xt_ps = psum_xt.tile([128, C], TRT)
                                for cs in range(CS):
                                    nc.tensor.transpose(
                                        xt_ps[:, cs * 128:(cs + 1) * 128],
                                        x_sb[:, cs, ns * 128:(ns + 1) * 128].bitcast(TRT),
                                        ident_r,
                                    )
                                mode = cfg["xt_copy"]
                                if mode == "act":
                                    use_act = True
                                elif mode == "dve":
                                    use_act = False
                                elif mode == "1of4":
                                    use_act = (ns % 4 == 0)
                                else:
                                    use_act = (ns % 2 == 0)
                                if use_act:
                                    nc.scalar.copy(xt_sb, xt_ps.bitcast(MMT))
                                else:
                                    nc.vector.tensor_copy(xt_sb, xt_ps.bitcast(MMT))

                            if not cfg["no_pool"]:
                                if cfg["pool_pack"]:
                                    half = ns % 2
                                    nc.tensor.matmul(
                                        tt_ps[half * M:(half + 1) * M, :],
                                        lhsT=elt_sb[:, ns, :],
                                        rhs=xt_sb,
                                        start=(ch == 0 and ns == half),
                                        stop=(ch == CH - 1 and ns == NS - 2 + half),
                                        skip_group_check=True,
                                        tile_position=(0, half * M),
                                    )
                                else:
                                    nc.tensor.matmul(
                                        tt_ps,
                                        lhsT=elt_sb[:, ns, :],
                                        rhs=xt_sb,
                                        start=(ch == 0 and ns == 0),
                                        stop=(ch == CH - 1 and ns == NS - 1),
                                        skip_group_check=True,
                                    )

                    tt_sb = ttpool.tile([M, C], F32)
                    if cfg["pool_pack"]:
                        nc.vector.tensor_tensor(
                            tt_sb, tt_ps[:M, :], tt_ps[M:, :], mybir.AluOpType.add)
                    else:
                        nc.scalar.copy(tt_sb, tt_ps)
                    (nc.scalar if cfg["st_dma_act"] else nc.sync).dma_start(tt_d[b], tt_sb)

    nc.compile()
    return nc


_NC = None


def _get_nc():
    global _NC
    if _NC is None:
        _NC = build_program()
    return _NC


def run(x, Vw, Vb, U, trace=False, trace_kwargs=None):
    x = np.ascontiguousarray(np.asarray(x, dtype=np.float32)).reshape(B, C, N)
    Vw = np.ascontiguousarray(np.asarray(Vw, dtype=np.float32))
    Vb = np.ascontiguousarray(np.asarray(Vb, dtype=np.float32)).reshape(R, 1)
    U = np.ascontiguousarray(np.asarray(U, dtype=np.float32))

    shards = x.reshape(NCORES, BS, C, N)
    in_maps = [
        {"x": np.ascontiguousarray(shards[i]), "Vw": Vw, "Vb": Vb, "U": U}
        for i in range(NCORES)
    ]
    nc = _get_nc()
    kw = {}
    if trace:
        kw["trace"] = True
        if trace_kwargs:
            kw["trace_kwargs"] = trace_kwargs
    res = bass_utils.run_bass_kernel_spmd(nc, in_maps, core_ids=list(range(NCORES)), **kw)
    tt = np.concatenate([r["TT"] for r in res.results], axis=0)  # [B, M, C]
    T = np.ascontiguousarray(tt.transpose(0, 2, 1))              # [B, C, M]
    st = np.concatenate([r["ST"] for r in res.results], axis=0)
    if DEFAULT_CFG["st_blocked"]:
        # st: [B, CH, 128, NS, M]; n = ch*512 + ns*128 + p
        S = np.ascontiguousarray(
            st.transpose(0, 4, 1, 3, 2).reshape(B, M, N))        # [B, M, N]
    else:
        S = np.ascontiguousarray(st.transpose(0, 2, 1))          # [B, M, N]
    return (T, S), res


def kernel(x, Vw, Vb, U):
    (T, S), _ = run(x, Vw, Vb, U, trace=False)
    return (T, S)


# ---------------------------------------------------------------------------
# Benchmarking helpers (not used by the grading path)
# ---------------------------------------------------------------------------

def _make_exec(nc, n_cores):
    """Mirror bass2jax.run_bass_via_pjrt's multi-core path, but return a jitted
    callable taking (inputs..., out_scratch...) with device-resident arrays, so
    repeated launches measure only kernel execution."""
    import jax
    from jax.experimental.shard_map import shard_map
    from jax.sharding import Mesh, NamedSharding, PartitionSpec
    from concourse import bass2jax

    bass2jax.install_neuronx_cc_hook()
    partition_name = nc.partition_id_tensor.name if nc.partition_id_tensor else None
    in_names, out_names, out_avals = [], [], []
    for alloc in nc.m.functions[0].allocations:
        if not isinstance(alloc, mybir.MemoryLocationSet):
            continue
        name = alloc.memorylocations[0].name
        if alloc.kind == "ExternalInput":
            if name != partition_name:
                in_names.append(name)
        elif alloc.kind == "ExternalOutput":
            out_names.append(name)
            out_avals.append(
                jax.core.ShapedArray(tuple(alloc.tensor_shape), mybir.dt.np(alloc.dtype))
            )
    n_params = len(in_names)
    n_outs = len(out_names)
    all_in = list(in_names) + list(out_names)
    if partition_name is not None:
        all_in.append(partition_name)

    def _body(*args):
        operands = list(args)
        if partition_name is not None:
            operands.append(bass2jax.partition_id_tensor())
        outs = bass2jax._bass_exec_p.bind(
            *operands,
            out_avals=tuple(out_avals),
            in_names=tuple(all_in),
            out_names=tuple(out_names),
            lowering_input_output_aliases=(),
            sim_require_finite=True,
            sim_require_nnan=True,
            nc=nc,
        )
        return tuple(outs)

    devices = jax.devices()[:n_cores]
    mesh = Mesh(np.asarray(devices), ("core",))
    spec = PartitionSpec("core")
    fn = jax.jit(
        shard_map(
            _body, mesh=mesh,
            in_specs=(spec,) * (n_params + n_outs),
            out_specs=(spec,) * n_outs,
            check_rep=False,
        ),
        donate_argnums=tuple(range(n_params, n_params + n_outs)),
        keep_unused=True,
    )
    sharding = NamedSharding(mesh, spec)
    return fn, in_names, out_names, out_avals, sharding


def bench(x, Vw, Vb, U, iters=20, warmup=3):
    """Measure steady-state per-launch wall time with device-resident inputs.
    Outputs of launch k are re-donated as scratch for launch k+1."""
    import time
    import jax

    x = np.ascontiguousarray(np.asarray(x, dtype=np.float32)).reshape(B, C, N)
    Vw = np.ascontiguousarray(np.asarray(Vw, dtype=np.float32))
    Vb = np.ascontiguousarray(np.asarray(Vb, dtype=np.float32)).reshape(R, 1)
    U = np.ascontiguousarray(np.asarray(U, dtype=np.float32))
    shards = x.reshape(NCORES, BS, C, N)

    nc = _get_nc()
    fn, in_names, out_names, out_avals, sharding = _make_exec(nc, NCORES)

    per_core = {
        "x": shards.reshape(NCORES * BS, C, N),
        "Vw": np.concatenate([Vw] * NCORES, 0),
        "Vb": np.concatenate([Vb] * NCORES, 0),
        "U": np.concatenate([U] * NCORES, 0),
    }
    in_dev = [jax.device_put(per_core[n], sharding) for n in in_names]
    zeros = [
        jax.device_put(np.zeros((NCORES * a.shape[0], *a.shape[1:]), a.dtype), sharding)
        for a in out_avals
    ]

    outs = fn(*in_dev, *list(zeros))
    jax.block_until_ready(outs)
    for _ in range(warmup - 1):
        outs = fn(*in_dev, *outs)
        jax.block_until_ready(outs)

    # pipelined timing
    t0 = time.perf_counter()
    for _ in range(iters):
        outs = fn(*in_dev, *outs)
    jax.block_until_ready(outs)
    t1 = time.perf_counter()
    pipelined_ns = (t1 - t0) / iters * 1e9

    # serial timing (per-launch incl round trip)
    t0 = time.perf_counter()
    for _ in range(iters):
        outs = fn(*in_dev, *outs)
        jax.block_until_ready(outs)
    t1 = time.perf_counter()
    serial_ns = (t1 - t0) / iters * 1e9

    return {"pipelined_ns": pipelined_ns, "serial_ns": serial_ns}
